# revision 20
# baseline (speedup 1.0000x reference)
"""Trainium2 Bass kernel: ApproxLayerNorm (q8.8 fixed-point layernorm with PWL
sqrt/reciprocal), data-parallel over 8 NeuronCores.

Self-contained: hardcodes shapes B=8192, D=4096, G=16, N_SEG=32.

Mirrors the int64 reference bit-for-bit (up to fp32 stat-accumulation noise)
using fp32 ops:
  x_q = round(x*256)          -- magic-constant round-to-nearest-even
  per-chunk (256) sums and M2 via one bn_stats per 512 block with an
  interleaved read AP: evens of the streamed order = chunk 2b, odds = 2b+1,
  so mean_e/M2_e are exactly chunk stats.
  Chan pairwise merge with the reference's integer floor-divisions emulated
  via exact fp32 floors; var q8.8 -> PWL sqrt -> PWL recip (searchsorted-right
  emulated by counting breaks <= v); out = (x_q - mu)/256 * inv * w + b.

Two build variants picked at run time from the weight/bias values:
  trivial (weight==1, bias==0): tail = xq*s + c (ACT Identity / DVE TS)
  general: tail = affine_mul_reduce (*w) + affine_then_add (+b) on DVE
"""

import numpy as np
from contextlib import ExitStack

import concourse.bass as bass
import concourse.tile as tile
from concourse import bacc, mybir
from concourse.bass_utils import run_bass_kernel_spmd

F32 = mybir.dt.float32
I16 = mybir.dt.int16
AF = mybir.ActivationFunctionType
OP = mybir.AluOpType
AX = mybir.AxisListType

B, D = 8192, 4096
N_CORES = 8
G = 16                 # variance chunks per row
CHUNK = D // G         # 256
BLK = 2 * CHUNK        # 512: one bn_stats block = chunk pair
NBLK = G // 2          # 8
N_SEG = 32
EPS = 1e-05
P = 128

MAGIC = 12582912.0     # 1.5*2^23: fp32 round-to-nearest-even magic
D256 = 0.5 - 1.0 / 512.0     # floor delta for 1/256-grid fractions
D4096 = 0.5 - 1.0 / 8192.0   # floor delta for 1/4096-grid fractions

# const-row layout (single [1, CONST_W] f32 input, broadcast to 128 partitions)
_SB, _SS, _SI = 0, 33, 65          # sqrt breaks/slopes/intercepts
_RB, _RS, _RI = 97, 130, 162       # recip breaks/slopes/intercepts
_IOTA = 194
_PROBE = 226
N_PROBE = 32
CONST_W = 258


def _floor_robust(nc, pool, y, shape, tag):
    """floor(y) for |y| < 2^22, any fraction: r=rn(y); r -= (r>y)."""
    r = pool.tile(shape, F32, tag=tag + "_r")
    nc.vector.tensor_scalar(out=r, in0=y, scalar1=MAGIC, scalar2=MAGIC,
                            op0=OP.add, op1=OP.subtract)
    gt = pool.tile(shape, F32, tag=tag + "_g")
    nc.vector.tensor_tensor(out=gt, in0=r, in1=y, op=OP.is_gt)
    nc.vector.tensor_tensor(out=r, in0=r, in1=gt, op=OP.subtract)
    return r


def _floor_delta(nc, pool, src, mul, delta, shape, tag):
    """floor(src*mul) when src*mul is exact and its fraction grid makes
    rn(src*mul - delta) == floor(src*mul) (no ties). 2 ops."""
    q = pool.tile(shape, F32, tag=tag + "_q")
    nc.vector.tensor_scalar(out=q, in0=src, scalar1=mul, scalar2=-delta,
                            op0=OP.mult, op1=OP.add)
    r = pool.tile(shape, F32, tag=tag + "_r")
    nc.vector.tensor_scalar(out=r, in0=q, scalar1=MAGIC, scalar2=MAGIC,
                            op0=OP.add, op1=OP.subtract)
    return r


def _pwl(nc, pool, v, csb, b_off, s_off, i_off, Tg, tag):
    """PWL table eval on [P, Tg] per-row scalars v (searchsorted-right)."""
    ge = pool.tile([P, Tg, N_SEG + 1], F32, tag=tag + "_ge")
    breaks_b = csb[:, b_off:b_off + N_SEG + 1].unsqueeze(1).broadcast_to(
        [P, Tg, N_SEG + 1])
    v_b = v.unsqueeze(2).broadcast_to([P, Tg, N_SEG + 1])
    nc.vector.tensor_tensor(out=ge, in0=breaks_b, in1=v_b, op=OP.is_le)
    cnt = pool.tile([P, Tg], F32, tag=tag + "_cnt")
    nc.vector.tensor_reduce(out=cnt, in_=ge, axis=AX.X, op=OP.add)
    idx = pool.tile([P, Tg], F32, tag=tag + "_idx")
    nc.vector.tensor_scalar(out=idx, in0=cnt, scalar1=-1.0, scalar2=0.0,
                            op0=OP.add, op1=OP.max)
    nc.vector.tensor_scalar(out=idx, in0=idx, scalar1=float(N_SEG - 1),
                            scalar2=None, op0=OP.min)
    oh = pool.tile([P, Tg, N_SEG], F32, tag=tag + "_oh")
    iota_b = csb[:, _IOTA:_IOTA + N_SEG].unsqueeze(1).broadcast_to([P, Tg, N_SEG])
    idx_b = idx.unsqueeze(2).broadcast_to([P, Tg, N_SEG])
    nc.vector.tensor_tensor(out=oh, in0=iota_b, in1=idx_b, op=OP.is_equal)
    slp_prod = pool.tile([P, Tg, N_SEG], F32, tag=tag + "_sp")
    slopes_b = csb[:, s_off:s_off + N_SEG].unsqueeze(1).broadcast_to([P, Tg, N_SEG])
    nc.vector.tensor_tensor(out=slp_prod, in0=oh, in1=slopes_b, op=OP.mult)
    slope = pool.tile([P, Tg], F32, tag=tag + "_sl")
    nc.vector.tensor_reduce(out=slope, in_=slp_prod, axis=AX.X, op=OP.add)
    icp_prod = pool.tile([P, Tg, N_SEG], F32, tag=tag + "_ip")
    iceps_b = csb[:, i_off:i_off + N_SEG].unsqueeze(1).broadcast_to([P, Tg, N_SEG])
    nc.vector.tensor_tensor(out=icp_prod, in0=oh, in1=iceps_b, op=OP.mult)
    icept = pool.tile([P, Tg], F32, tag=tag + "_ic")
    nc.vector.tensor_reduce(out=icept, in_=icp_prod, axis=AX.X, op=OP.add)
    out = pool.tile([P, Tg], F32, tag=tag + "_out")
    nc.vector.tensor_tensor(out=out, in0=slope, in1=v, op=OP.mult)
    nc.vector.tensor_tensor(out=out, in0=out, in1=icept, op=OP.add)
    msk = pool.tile([P, Tg], F32, tag=tag + "_mk")
    nc.vector.tensor_scalar(out=msk, in0=cnt, scalar1=1.0, scalar2=None,
                            op0=OP.is_ge)
    nc.vector.tensor_tensor(out=out, in0=out, in1=msk, op=OP.mult)
    return out


def _phase2(nc, pool, csb, stats, Tg, gname):
    """stats [P, Tg, G, 6] (per-chunk bn_stats, even/odd = half-chunks)
    -> (s_pp, c_pp) [P, Tg] each."""
    mean_e = stats[:, :, :, 1]
    m2_e = stats[:, :, :, 2]
    mean_o = stats[:, :, :, 4]
    m2_o = stats[:, :, :, 5]
    sh = [P, Tg, G]

    # S_g/128 = mean_e + mean_o (exact); m_g = floor(S_g/256)
    msum = pool.tile(sh, F32, tag=gname + "msum")
    nc.vector.tensor_tensor(out=msum, in0=mean_e, in1=mean_o, op=OP.add)
    m_g = _floor_delta(nc, pool, msum, 0.5, D256, sh, gname + "mg")

    # M_g = M2_e + M2_o + 128*((mean_e-m)^2 + (mean_o-m)^2)
    dm_e = pool.tile(sh, F32, tag=gname + "dme")
    nc.vector.tensor_tensor(out=dm_e, in0=mean_e, in1=m_g, op=OP.subtract)
    dm_o = pool.tile(sh, F32, tag=gname + "dmo")
    nc.vector.tensor_tensor(out=dm_o, in0=mean_o, in1=m_g, op=OP.subtract)
    nc.vector.tensor_tensor(out=dm_e, in0=dm_e, in1=dm_e, op=OP.mult)
    nc.vector.tensor_tensor(out=dm_o, in0=dm_o, in1=dm_o, op=OP.mult)
    nc.vector.tensor_tensor(out=dm_e, in0=dm_e, in1=dm_o, op=OP.add)
    M_cur = pool.tile(sh, F32, tag=gname + "Mg")
    nc.vector.tensor_tensor(out=M_cur, in0=m2_e, in1=m2_o, op=OP.add)
    nc.vector.scalar_tensor_tensor(out=M_cur, in0=dm_e,
                                   scalar=float(CHUNK // 2), in1=M_cur,
                                   op0=OP.mult, op1=OP.add)
    m_cur = m_g

    # row mean: mu = floor(S_row/D); reduce(msum) = S_row/128
    gsum = pool.tile([P, Tg], F32, tag=gname + "gsum")
    nc.vector.tensor_reduce(out=gsum, in_=msum, axis=AX.X, op=OP.add)
    mu_row = _floor_delta(nc, pool, gsum, 1.0 / (D // P), D4096,
                          [P, Tg], gname + "mu")

    width, n_cur, lvl = G, CHUNK, 1
    while width > 1:
        w2 = width // 2
        m0 = m_cur[:, :, 0:width:2]
        m1 = m_cur[:, :, 1:width:2]
        M0 = M_cur[:, :, 0:width:2]
        M1 = M_cur[:, :, 1:width:2]
        d = pool.tile([P, Tg, w2], F32, tag=f"{gname}d{lvl}")
        nc.vector.tensor_tensor(out=d, in0=m0, in1=m1, op=OP.subtract)
        nc.vector.tensor_tensor(out=d, in0=d, in1=d, op=OP.mult)
        Mn = pool.tile([P, Tg, w2], F32, tag=f"{gname}M{lvl}")
        nc.vector.tensor_tensor(out=Mn, in0=M0, in1=M1, op=OP.add)
        nc.vector.scalar_tensor_tensor(out=Mn, in0=d, scalar=float(n_cur // 2),
                                       in1=Mn, op0=OP.mult, op1=OP.add)
        ms = pool.tile([P, Tg, w2], F32, tag=f"{gname}ms{lvl}")
        nc.vector.tensor_tensor(out=ms, in0=m0, in1=m1, op=OP.add)
        m_cur = _floor_delta(nc, pool, ms, 0.5, 0.25, [P, Tg, w2],
                             f"{gname}mn{lvl}")
        M_cur, width, n_cur = Mn, w2, n_cur * 2
        lvl += 1

    Mfin = M_cur.squeeze(2)
    y16 = pool.tile([P, Tg], F32, tag=gname + "y16")
    nc.vector.tensor_scalar(out=y16, in0=Mfin, scalar1=1.0 / D, scalar2=None,
                            op0=OP.mult)
    v16 = _floor_robust(nc, pool, y16, [P, Tg], gname + "v16")
    v8 = _floor_delta(nc, pool, v16, 1.0 / 256.0, D256, [P, Tg], gname + "v8")
    v1 = pool.tile([P, Tg], F32, tag=gname + "v1")
    nc.vector.tensor_scalar(out=v1, in0=v8, scalar1=1.0 / 256.0, scalar2=EPS,
                            op0=OP.mult, op1=OP.add)

    sqrt_v = _pwl(nc, pool, v1, csb, _SB, _SS, _SI, Tg, gname + "sq")
    inv = _pwl(nc, pool, sqrt_v, csb, _RB, _RS, _RI, Tg, gname + "rc")

    s_pp = pool.tile([P, Tg], F32, tag=gname + "s")
    nc.vector.tensor_scalar(out=s_pp, in0=inv, scalar1=1.0 / 256.0, scalar2=None,
                            op0=OP.mult)
    c_pp = pool.tile([P, Tg], F32, tag=gname + "c")
    nc.vector.scalar_tensor_tensor(out=c_pp, in0=mu_row, scalar=-1.0, in1=s_pp,
                                   op0=OP.mult, op1=OP.mult)
    return s_pp, c_pp


def build_kernel(ctx: ExitStack, tc: tile.TileContext, ntiles: int, trivial: bool,
                 x_dram, w_dram, b_dram, c_dram, out_dram, probe_dram):
    nc = tc.nc
    T = ntiles

    singles = ctx.enter_context(tc.tile_pool(name="singles", bufs=1))
    xin_pool = ctx.enter_context(tc.tile_pool(name="xin", bufs=4))
    xq_pool = ctx.enter_context(tc.tile_pool(name="xq", bufs=1))
    small = ctx.enter_context(tc.tile_pool(name="small", bufs=1))
    tails = ctx.enter_context(tc.tile_pool(name="tails", bufs=2))

    # ---- grouped pipeline: big first group, tiny last group so the final
    # phase-2 -> tail -> store chain is short ----
    groups = [list(range(T))] if T <= 2 else [
        list(range(0, (T * 3) // 4)), list(range((T * 3) // 4, T))]

    # issue the first group's x loads before anything else so DMA ramps
    # immediately; constants ride behind them
    xin_tiles = {}
    half = D // 2
    for t in groups[0]:
        xin = xin_pool.tile([P, D], F32, tag="xin")
        xin_tiles[t] = xin
        nc.sync.dma_start(out=xin[:, 0:half],
                          in_=x_dram[t * P:(t + 1) * P, 0:half])
        nc.sync.dma_start(out=xin[:, half:D],
                          in_=x_dram[t * P:(t + 1) * P, half:D])

    # ---- constants ----
    csb = singles.tile([P, CONST_W], F32)
    nc.sync.dma_start(out=csb, in_=c_dram[0:1, :].partition_broadcast(P).squeeze(1))
    if not trivial:
        w_rep = singles.tile([P, D], F32)
        nc.sync.dma_start(out=w_rep,
                          in_=w_dram[0:1, :].partition_broadcast(P).squeeze(1))
        b_rep = singles.tile([P, D], F32)
        nc.sync.dma_start(out=b_rep,
                          in_=b_dram[0:1, :].partition_broadcast(P).squeeze(1))

    # ---- int16-convert rounding-mode probe (row0: DVE, row1: ACT) ----
    pr0 = singles.tile([1, N_PROBE], I16)
    nc.vector.tensor_scalar(out=pr0, in0=csb[0:1, _PROBE:_PROBE + N_PROBE],
                            scalar1=1.0, scalar2=None, op0=OP.mult)
    pr1 = singles.tile([1, N_PROBE], I16)
    nc.scalar.activation(out=pr1, in_=csb[0:1, _PROBE:_PROBE + N_PROBE],
                         func=AF.Copy, bias=0.0, scale=1.0)
    nc.sync.dma_start(out=probe_dram[0:1, :], in_=pr0)
    nc.sync.dma_start(out=probe_dram[1:2, :], in_=pr1)

    for gi, tlist in enumerate(groups):
        Tg = len(tlist)
        gname = f"g{gi}"
        stats = singles.tile([P, Tg, G, 6], F32, tag=gname + "stats")
        xq_tiles = {}
        for j, t in enumerate(tlist):
            if t in xin_tiles:
                xin = xin_tiles.pop(t)
            else:
                xin = xin_pool.tile([P, D], F32, tag="xin")
                nc.sync.dma_start(out=xin[:, 0:half],
                                  in_=x_dram[t * P:(t + 1) * P, 0:half])
                nc.sync.dma_start(out=xin[:, half:D],
                                  in_=x_dram[t * P:(t + 1) * P, half:D])
            xq = xq_pool.tile([P, D], I16, tag=f"xq{t}")
            xq_tiles[t] = xq
            # round to q8.8 codes in ONE op: the fp32->int16 write converter
            # rounds to nearest-even (probe-verified on HW), matching
            # jnp.round(x*256) exactly; per-half ops pipeline with the DMA
            nc.scalar.activation(out=xq[:, 0:half], in_=xin[:, 0:half],
                                 func=AF.Copy, bias=0.0, scale=256.0)
            nc.scalar.activation(out=xq[:, half:D], in_=xin[:, half:D],
                                 func=AF.Copy, bias=0.0, scale=256.0)
            # per-chunk stats: 16 ops x [P, 256] -> [P, 6]
            for c in range(G):
                nc.vector.bn_stats(out=stats[:, j, c, :],
                                   in_=xq[:, c * CHUNK:(c + 1) * CHUNK])

        s_pp, c_pp = _phase2(nc, small, csb, stats, Tg, gname)

        # ---- tails ----
        if trivial:
            for j, t in enumerate(tlist):
                osb = tails.tile([P, D], F32, tag="osb")
                if t % 4 < 3:
                    nc.scalar.activation(out=osb, in_=xq_tiles[t],
                                         func=AF.Identity,
                                         bias=c_pp[:, j:j + 1],
                                         scale=s_pp[:, j:j + 1])
                else:
                    nc.vector.tensor_scalar(out=osb, in0=xq_tiles[t],
                                            scalar1=s_pp[:, j:j + 1],
                                            scalar2=c_pp[:, j:j + 1],
                                            op0=OP.mult, op1=OP.add)
                Q4 = D // 4
                for q in range(4):
                    nc.sync.dma_start(
                        out=out_dram[t * P:(t + 1) * P, q * Q4:(q + 1) * Q4],
                        in_=osb[:, q * Q4:(q + 1) * Q4])
        else:
            HALF = D // 2
            scr_pool = ctx.enter_context(
                tc.tile_pool(name=gname + "scr", bufs=4))
            for j, t in enumerate(tlist):
                xq = xq_tiles[t]
                for h in range(2):
                    col0 = h * HALF
                    xnw = tails.tile([P, HALF], F32, tag="xnw")
                    scr = scr_pool.tile([P, 1], F32, tag="scr")
                    nc.vector.affine_mul_reduce(
                        out=xnw, accum_out=scr,
                        in0=xq[:, col0:col0 + HALF],
                        in1=w_rep[:, col0:col0 + HALF],
                        scale=s_pp[:, j:j + 1], bias=c_pp[:, j:j + 1])
                    osb = tails.tile([P, HALF], F32, tag="osb")
                    nc.vector.affine_then_add(out=osb, in0=xnw,
                                              in1=b_rep[:, col0:col0 + HALF],
                                              scale=1.0, bias=0.0)
                    nc.sync.dma_start(out=out_dram[t * P:(t + 1) * P,
                                                   col0:col0 + HALF], in_=osb)


def build_nc(rows_per_core: int, trivial: bool):
    assert rows_per_core % P == 0
    ntiles = rows_per_core // P
    nc = bacc.Bacc("TRN2", target_bir_lowering=False, debug=False,
                   num_devices=N_CORES)
    x = nc.dram_tensor("x", [rows_per_core, D], F32, kind="ExternalInput").ap()
    if trivial:
        w = b = None
    else:
        w = nc.dram_tensor("weight", [1, D], F32, kind="ExternalInput").ap()
        b = nc.dram_tensor("bias", [1, D], F32, kind="ExternalInput").ap()
    c = nc.dram_tensor("consts", [1, CONST_W], F32, kind="ExternalInput").ap()
    out = nc.dram_tensor("out", [rows_per_core, D], F32, kind="ExternalOutput").ap()
    probe = nc.dram_tensor("probe", [2, N_PROBE], I16, kind="ExternalOutput").ap()
    with tile.TileContext(nc) as tc, ExitStack() as ctx:
        build_kernel(ctx, tc, ntiles, trivial, x, w, b, c, out, probe)
    nc.compile()
    return nc


_PROBE_VALS = np.array(
    [2.5, 3.5, -2.5, -3.5, 0.5, 1.5, -0.5, -1.5,
     2.7, -2.7, 2.3, -2.3, 0.4999, -0.4999, 100.5, -100.5,
     7.5, -7.5, 8.5, -8.5, 1.0, -1.0, 0.0, 12.25,
     -12.25, 3.49999, -3.49999, 255.5, -255.5, 1023.5, 33.5, -33.5],
    np.float32)


def make_consts(sqrt_breaks, sqrt_slopes, sqrt_intercepts,
                recip_breaks, recip_slopes, recip_intercepts):
    c = np.zeros((1, CONST_W), np.float32)
    c[0, _SB:_SB + 33] = sqrt_breaks
    c[0, _SS:_SS + 32] = sqrt_slopes
    c[0, _SI:_SI + 32] = sqrt_intercepts
    c[0, _RB:_RB + 33] = recip_breaks
    c[0, _RS:_RS + 32] = recip_slopes
    c[0, _RI:_RI + 32] = recip_intercepts
    c[0, _IOTA:_IOTA + 32] = np.arange(32, dtype=np.float32)
    c[0, _PROBE:_PROBE + N_PROBE] = _PROBE_VALS
    return c


_NC_CACHE = {}


def _get_nc(rows_per_core, trivial):
    key = (rows_per_core, trivial)
    if key not in _NC_CACHE:
        _NC_CACHE[key] = build_nc(rows_per_core, trivial)
    return _NC_CACHE[key]


def run(x, weight, bias, consts, trace=False, **trace_kwargs):
    rows = x.shape[0] // N_CORES
    weight = np.asarray(weight, np.float32).reshape(1, D)
    bias = np.asarray(bias, np.float32).reshape(1, D)
    trivial = bool(np.all(weight == 1.0) and np.all(bias == 0.0))
    nc = _get_nc(rows, trivial)
    in_maps = []
    for i in range(N_CORES):
        m = {"x": np.ascontiguousarray(x[i * rows:(i + 1) * rows],
                                       dtype=np.float32),
             "consts": consts}
        if not trivial:
            m["weight"] = weight
            m["bias"] = bias
        in_maps.append(m)
    res = run_bass_kernel_spmd(nc, in_maps, core_ids=list(range(N_CORES)),
                               trace=trace, **trace_kwargs)
    out = np.concatenate([r["out"] for r in res.results], axis=0)
    return out, res


def kernel(x, weight, bias, sqrt_breaks, sqrt_slopes, sqrt_intercepts,
           recip_breaks, recip_slopes, recip_intercepts):
    x = np.asarray(x, dtype=np.float32)
    consts = make_consts(np.asarray(sqrt_breaks), np.asarray(sqrt_slopes),
                         np.asarray(sqrt_intercepts), np.asarray(recip_breaks),
                         np.asarray(recip_slopes), np.asarray(recip_intercepts))
    out, _ = run(x, np.asarray(weight), np.asarray(bias), consts, trace=False)
    return out


# revision 21
# speedup vs baseline: 1.1627x; 1.1627x over previous
"""Trainium2 Bass kernel: ApproxLayerNorm (q8.8 fixed-point layernorm with PWL
sqrt/reciprocal), data-parallel over 8 NeuronCores.

Self-contained: hardcodes shapes B=8192, D=4096, G=16, N_SEG=32.

Mirrors the int64 reference bit-for-bit (up to fp32 stat-accumulation noise)
using fp32 ops:
  x_q = round(x*256)          -- magic-constant round-to-nearest-even
  per-chunk (256) sums and M2 via one bn_stats per 512 block with an
  interleaved read AP: evens of the streamed order = chunk 2b, odds = 2b+1,
  so mean_e/M2_e are exactly chunk stats.
  Chan pairwise merge with the reference's integer floor-divisions emulated
  via exact fp32 floors; var q8.8 -> PWL sqrt -> PWL recip (searchsorted-right
  emulated by counting breaks <= v); out = (x_q - mu)/256 * inv * w + b.

Two build variants picked at run time from the weight/bias values:
  trivial (weight==1, bias==0): tail = xq*s + c (ACT Identity / DVE TS)
  general: tail = affine_mul_reduce (*w) + affine_then_add (+b) on DVE
"""

import numpy as np
from contextlib import ExitStack

import concourse.bass as bass
import concourse.tile as tile
from concourse import bacc, mybir
from concourse.bass_utils import run_bass_kernel_spmd

F32 = mybir.dt.float32
I16 = mybir.dt.int16
AF = mybir.ActivationFunctionType
OP = mybir.AluOpType
AX = mybir.AxisListType

B, D = 8192, 4096
N_CORES = 8
G = 16                 # variance chunks per row
CHUNK = D // G         # 256
BLK = 2 * CHUNK        # 512: one bn_stats block = chunk pair
NBLK = G // 2          # 8
N_SEG = 32
EPS = 1e-05
P = 128

MAGIC = 12582912.0     # 1.5*2^23: fp32 round-to-nearest-even magic
D256 = 0.5 - 1.0 / 512.0     # floor delta for 1/256-grid fractions
D4096 = 0.5 - 1.0 / 8192.0   # floor delta for 1/4096-grid fractions

# const-row layout (single [1, CONST_W] f32 input, broadcast to 128 partitions)
_SB, _SS, _SI = 0, 33, 65          # sqrt breaks/slopes/intercepts
_RB, _RS, _RI = 97, 130, 162       # recip breaks/slopes/intercepts
_IOTA = 194
_PROBE = 226
N_PROBE = 32
CONST_W = 258


def _floor_robust(nc, pool, y, shape, tag):
    """floor(y) for |y| < 2^22, any fraction: r=rn(y); r -= (r>y)."""
    r = pool.tile(shape, F32, tag=tag + "_r")
    nc.vector.tensor_scalar(out=r, in0=y, scalar1=MAGIC, scalar2=MAGIC,
                            op0=OP.add, op1=OP.subtract)
    gt = pool.tile(shape, F32, tag=tag + "_g")
    nc.vector.tensor_tensor(out=gt, in0=r, in1=y, op=OP.is_gt)
    nc.vector.tensor_tensor(out=r, in0=r, in1=gt, op=OP.subtract)
    return r


def _floor_delta(nc, pool, src, mul, delta, shape, tag):
    """floor(src*mul) when src*mul is exact and its fraction grid makes
    rn(src*mul - delta) == floor(src*mul) (no ties). 2 ops."""
    q = pool.tile(shape, F32, tag=tag + "_q")
    nc.vector.tensor_scalar(out=q, in0=src, scalar1=mul, scalar2=-delta,
                            op0=OP.mult, op1=OP.add)
    r = pool.tile(shape, F32, tag=tag + "_r")
    nc.vector.tensor_scalar(out=r, in0=q, scalar1=MAGIC, scalar2=MAGIC,
                            op0=OP.add, op1=OP.subtract)
    return r


def _pwl(nc, pool, v, csb, b_off, s_off, i_off, Tg, tag):
    """PWL table eval on [P, Tg] per-row scalars v (searchsorted-right)."""
    ge = pool.tile([P, Tg, N_SEG + 1], F32, tag=tag + "_ge")
    breaks_b = csb[:, b_off:b_off + N_SEG + 1].unsqueeze(1).broadcast_to(
        [P, Tg, N_SEG + 1])
    v_b = v.unsqueeze(2).broadcast_to([P, Tg, N_SEG + 1])
    nc.vector.tensor_tensor(out=ge, in0=breaks_b, in1=v_b, op=OP.is_le)
    cnt = pool.tile([P, Tg], F32, tag=tag + "_cnt")
    nc.vector.tensor_reduce(out=cnt, in_=ge, axis=AX.X, op=OP.add)
    idx = pool.tile([P, Tg], F32, tag=tag + "_idx")
    nc.vector.tensor_scalar(out=idx, in0=cnt, scalar1=-1.0, scalar2=0.0,
                            op0=OP.add, op1=OP.max)
    nc.vector.tensor_scalar(out=idx, in0=idx, scalar1=float(N_SEG - 1),
                            scalar2=None, op0=OP.min)
    oh = pool.tile([P, Tg, N_SEG], F32, tag=tag + "_oh")
    iota_b = csb[:, _IOTA:_IOTA + N_SEG].unsqueeze(1).broadcast_to([P, Tg, N_SEG])
    idx_b = idx.unsqueeze(2).broadcast_to([P, Tg, N_SEG])
    nc.vector.tensor_tensor(out=oh, in0=iota_b, in1=idx_b, op=OP.is_equal)
    slp_prod = pool.tile([P, Tg, N_SEG], F32, tag=tag + "_sp")
    slopes_b = csb[:, s_off:s_off + N_SEG].unsqueeze(1).broadcast_to([P, Tg, N_SEG])
    nc.vector.tensor_tensor(out=slp_prod, in0=oh, in1=slopes_b, op=OP.mult)
    slope = pool.tile([P, Tg], F32, tag=tag + "_sl")
    nc.vector.tensor_reduce(out=slope, in_=slp_prod, axis=AX.X, op=OP.add)
    icp_prod = pool.tile([P, Tg, N_SEG], F32, tag=tag + "_ip")
    iceps_b = csb[:, i_off:i_off + N_SEG].unsqueeze(1).broadcast_to([P, Tg, N_SEG])
    nc.vector.tensor_tensor(out=icp_prod, in0=oh, in1=iceps_b, op=OP.mult)
    icept = pool.tile([P, Tg], F32, tag=tag + "_ic")
    nc.vector.tensor_reduce(out=icept, in_=icp_prod, axis=AX.X, op=OP.add)
    out = pool.tile([P, Tg], F32, tag=tag + "_out")
    nc.vector.tensor_tensor(out=out, in0=slope, in1=v, op=OP.mult)
    nc.vector.tensor_tensor(out=out, in0=out, in1=icept, op=OP.add)
    msk = pool.tile([P, Tg], F32, tag=tag + "_mk")
    nc.vector.tensor_scalar(out=msk, in0=cnt, scalar1=1.0, scalar2=None,
                            op0=OP.is_ge)
    nc.vector.tensor_tensor(out=out, in0=out, in1=msk, op=OP.mult)
    return out


def _phase2(nc, pool, csb, stats, Tg, gname):
    """stats [P, Tg, G, 6] (per-chunk bn_stats, even/odd = half-chunks)
    -> (s_pp, c_pp) [P, Tg] each."""
    mean_e = stats[:, :, :, 1]
    m2_e = stats[:, :, :, 2]
    mean_o = stats[:, :, :, 4]
    m2_o = stats[:, :, :, 5]
    sh = [P, Tg, G]

    # S_g/128 = mean_e + mean_o (exact); m_g = floor(S_g/256)
    msum = pool.tile(sh, F32, tag=gname + "msum")
    nc.vector.tensor_tensor(out=msum, in0=mean_e, in1=mean_o, op=OP.add)
    m_g = _floor_delta(nc, pool, msum, 0.5, D256, sh, gname + "mg")

    # M_g = M2_e + M2_o + 128*((mean_e-m)^2 + (mean_o-m)^2)
    dm_e = pool.tile(sh, F32, tag=gname + "dme")
    nc.vector.tensor_tensor(out=dm_e, in0=mean_e, in1=m_g, op=OP.subtract)
    dm_o = pool.tile(sh, F32, tag=gname + "dmo")
    nc.vector.tensor_tensor(out=dm_o, in0=mean_o, in1=m_g, op=OP.subtract)
    nc.vector.tensor_tensor(out=dm_e, in0=dm_e, in1=dm_e, op=OP.mult)
    nc.vector.tensor_tensor(out=dm_o, in0=dm_o, in1=dm_o, op=OP.mult)
    nc.vector.tensor_tensor(out=dm_e, in0=dm_e, in1=dm_o, op=OP.add)
    M_cur = pool.tile(sh, F32, tag=gname + "Mg")
    nc.vector.tensor_tensor(out=M_cur, in0=m2_e, in1=m2_o, op=OP.add)
    nc.vector.scalar_tensor_tensor(out=M_cur, in0=dm_e,
                                   scalar=float(CHUNK // 2), in1=M_cur,
                                   op0=OP.mult, op1=OP.add)
    m_cur = m_g

    # row mean: mu = floor(S_row/D); reduce(msum) = S_row/128
    gsum = pool.tile([P, Tg], F32, tag=gname + "gsum")
    nc.vector.tensor_reduce(out=gsum, in_=msum, axis=AX.X, op=OP.add)
    mu_row = _floor_delta(nc, pool, gsum, 1.0 / (D // P), D4096,
                          [P, Tg], gname + "mu")

    width, n_cur, lvl = G, CHUNK, 1
    while width > 1:
        w2 = width // 2
        m0 = m_cur[:, :, 0:width:2]
        m1 = m_cur[:, :, 1:width:2]
        M0 = M_cur[:, :, 0:width:2]
        M1 = M_cur[:, :, 1:width:2]
        d = pool.tile([P, Tg, w2], F32, tag=f"{gname}d{lvl}")
        nc.vector.tensor_tensor(out=d, in0=m0, in1=m1, op=OP.subtract)
        nc.vector.tensor_tensor(out=d, in0=d, in1=d, op=OP.mult)
        Mn = pool.tile([P, Tg, w2], F32, tag=f"{gname}M{lvl}")
        nc.vector.tensor_tensor(out=Mn, in0=M0, in1=M1, op=OP.add)
        nc.vector.scalar_tensor_tensor(out=Mn, in0=d, scalar=float(n_cur // 2),
                                       in1=Mn, op0=OP.mult, op1=OP.add)
        ms = pool.tile([P, Tg, w2], F32, tag=f"{gname}ms{lvl}")
        nc.vector.tensor_tensor(out=ms, in0=m0, in1=m1, op=OP.add)
        m_cur = _floor_delta(nc, pool, ms, 0.5, 0.25, [P, Tg, w2],
                             f"{gname}mn{lvl}")
        M_cur, width, n_cur = Mn, w2, n_cur * 2
        lvl += 1

    Mfin = M_cur.squeeze(2)
    y16 = pool.tile([P, Tg], F32, tag=gname + "y16")
    nc.vector.tensor_scalar(out=y16, in0=Mfin, scalar1=1.0 / D, scalar2=None,
                            op0=OP.mult)
    v16 = _floor_robust(nc, pool, y16, [P, Tg], gname + "v16")
    v8 = _floor_delta(nc, pool, v16, 1.0 / 256.0, D256, [P, Tg], gname + "v8")
    v1 = pool.tile([P, Tg], F32, tag=gname + "v1")
    nc.vector.tensor_scalar(out=v1, in0=v8, scalar1=1.0 / 256.0, scalar2=EPS,
                            op0=OP.mult, op1=OP.add)

    sqrt_v = _pwl(nc, pool, v1, csb, _SB, _SS, _SI, Tg, gname + "sq")
    inv = _pwl(nc, pool, sqrt_v, csb, _RB, _RS, _RI, Tg, gname + "rc")

    s_pp = pool.tile([P, Tg], F32, tag=gname + "s")
    nc.vector.tensor_scalar(out=s_pp, in0=inv, scalar1=1.0 / 256.0, scalar2=None,
                            op0=OP.mult)
    c_pp = pool.tile([P, Tg], F32, tag=gname + "c")
    nc.vector.scalar_tensor_tensor(out=c_pp, in0=mu_row, scalar=-1.0, in1=s_pp,
                                   op0=OP.mult, op1=OP.mult)
    return s_pp, c_pp


def build_kernel(ctx: ExitStack, tc: tile.TileContext, ntiles: int, trivial: bool,
                 x_dram, w_dram, b_dram, c_dram, out_dram, probe_dram):
    nc = tc.nc
    T = ntiles

    singles = ctx.enter_context(tc.tile_pool(name="singles", bufs=1))
    xin_pool = ctx.enter_context(tc.tile_pool(name="xin", bufs=4))
    xq_pool = ctx.enter_context(tc.tile_pool(name="xq", bufs=1))
    small = ctx.enter_context(tc.tile_pool(name="small", bufs=1))
    tails = ctx.enter_context(tc.tile_pool(name="tails", bufs=3))

    # ---- grouped pipeline: big first group, tiny last group so the final
    # phase-2 -> tail -> store chain is short ----
    groups = [list(range(T))] if T <= 2 else [
        list(range(0, T // 2)), list(range(T // 2, T))]

    # issue the first group's x loads before anything else so DMA ramps
    # immediately; constants ride behind them
    xin_tiles = {}
    half = D // 2
    for t in groups[0]:
        xin = xin_pool.tile([P, D], F32, tag="xin")
        xin_tiles[t] = xin
        nc.sync.dma_start(out=xin[:, 0:half],
                          in_=x_dram[t * P:(t + 1) * P, 0:half])
        nc.sync.dma_start(out=xin[:, half:D],
                          in_=x_dram[t * P:(t + 1) * P, half:D])

    # ---- constants ----
    csb = singles.tile([P, CONST_W], F32)
    nc.sync.dma_start(out=csb, in_=c_dram[0:1, :].partition_broadcast(P).squeeze(1))
    if not trivial:
        w_rep = singles.tile([P, D], F32)
        nc.sync.dma_start(out=w_rep,
                          in_=w_dram[0:1, :].partition_broadcast(P).squeeze(1))
        b_rep = singles.tile([P, D], F32)
        nc.sync.dma_start(out=b_rep,
                          in_=b_dram[0:1, :].partition_broadcast(P).squeeze(1))

    # ---- int16-convert rounding-mode probe (row0: DVE, row1: ACT) ----
    pr0 = singles.tile([1, N_PROBE], I16)
    nc.vector.tensor_scalar(out=pr0, in0=csb[0:1, _PROBE:_PROBE + N_PROBE],
                            scalar1=1.0, scalar2=None, op0=OP.mult)
    pr1 = singles.tile([1, N_PROBE], I16)
    nc.scalar.activation(out=pr1, in_=csb[0:1, _PROBE:_PROBE + N_PROBE],
                         func=AF.Copy, bias=0.0, scale=1.0)
    nc.sync.dma_start(out=probe_dram[0:1, :], in_=pr0)
    nc.sync.dma_start(out=probe_dram[1:2, :], in_=pr1)

    for gi, tlist in enumerate(groups):
        Tg = len(tlist)
        gname = f"g{gi}"
        stats = singles.tile([P, Tg, G, 6], F32, tag=gname + "stats")
        xq_tiles = {}
        for j, t in enumerate(tlist):
            if t in xin_tiles:
                xin = xin_tiles.pop(t)
            else:
                xin = xin_pool.tile([P, D], F32, tag="xin")
                nc.sync.dma_start(out=xin[:, 0:half],
                                  in_=x_dram[t * P:(t + 1) * P, 0:half])
                nc.sync.dma_start(out=xin[:, half:D],
                                  in_=x_dram[t * P:(t + 1) * P, half:D])
            xq = xq_pool.tile([P, D], I16, tag=f"xq{t}")
            xq_tiles[t] = xq
            # round to q8.8 codes in ONE op: the fp32->int16 write converter
            # rounds to nearest-even (probe-verified on HW), matching
            # jnp.round(x*256) exactly; per-half ops pipeline with the DMA
            nc.scalar.activation(out=xq, in_=xin, func=AF.Copy,
                                 bias=0.0, scale=256.0)
            # per-chunk stats: 16 ops x [P, 256] -> [P, 6]
            for c in range(G):
                nc.vector.bn_stats(out=stats[:, j, c, :],
                                   in_=xq[:, c * CHUNK:(c + 1) * CHUNK])

        s_pp, c_pp = _phase2(nc, small, csb, stats, Tg, gname)

        # ---- tails ----
        if trivial:
            for j, t in enumerate(tlist):
                osb = tails.tile([P, D], F32, tag="osb")
                if j % 2 == 0:
                    nc.scalar.activation(out=osb, in_=xq_tiles[t],
                                         func=AF.Identity,
                                         bias=c_pp[:, j:j + 1],
                                         scale=s_pp[:, j:j + 1])
                else:
                    nc.vector.tensor_scalar(out=osb, in0=xq_tiles[t],
                                            scalar1=s_pp[:, j:j + 1],
                                            scalar2=c_pp[:, j:j + 1],
                                            op0=OP.mult, op1=OP.add)
                nc.sync.dma_start(out=out_dram[t * P:(t + 1) * P, 0:half],
                                  in_=osb[:, 0:half])
                nc.sync.dma_start(out=out_dram[t * P:(t + 1) * P, half:D],
                                  in_=osb[:, half:D])
        else:
            HALF = D // 2
            scr_pool = ctx.enter_context(
                tc.tile_pool(name=gname + "scr", bufs=4))
            for j, t in enumerate(tlist):
                xq = xq_tiles[t]
                for h in range(2):
                    col0 = h * HALF
                    xnw = tails.tile([P, HALF], F32, tag="xnw")
                    scr = scr_pool.tile([P, 1], F32, tag="scr")
                    nc.vector.affine_mul_reduce(
                        out=xnw, accum_out=scr,
                        in0=xq[:, col0:col0 + HALF],
                        in1=w_rep[:, col0:col0 + HALF],
                        scale=s_pp[:, j:j + 1], bias=c_pp[:, j:j + 1])
                    osb = tails.tile([P, HALF], F32, tag="osb")
                    nc.vector.affine_then_add(out=osb, in0=xnw,
                                              in1=b_rep[:, col0:col0 + HALF],
                                              scale=1.0, bias=0.0)
                    nc.sync.dma_start(out=out_dram[t * P:(t + 1) * P,
                                                   col0:col0 + HALF], in_=osb)


def build_nc(rows_per_core: int, trivial: bool):
    assert rows_per_core % P == 0
    ntiles = rows_per_core // P
    nc = bacc.Bacc("TRN2", target_bir_lowering=False, debug=False,
                   num_devices=N_CORES)
    x = nc.dram_tensor("x", [rows_per_core, D], F32, kind="ExternalInput").ap()
    if trivial:
        w = b = None
    else:
        w = nc.dram_tensor("weight", [1, D], F32, kind="ExternalInput").ap()
        b = nc.dram_tensor("bias", [1, D], F32, kind="ExternalInput").ap()
    c = nc.dram_tensor("consts", [1, CONST_W], F32, kind="ExternalInput").ap()
    out = nc.dram_tensor("out", [rows_per_core, D], F32, kind="ExternalOutput").ap()
    probe = nc.dram_tensor("probe", [2, N_PROBE], I16, kind="ExternalOutput").ap()
    with tile.TileContext(nc) as tc, ExitStack() as ctx:
        build_kernel(ctx, tc, ntiles, trivial, x, w, b, c, out, probe)
    nc.compile()
    return nc


_PROBE_VALS = np.array(
    [2.5, 3.5, -2.5, -3.5, 0.5, 1.5, -0.5, -1.5,
     2.7, -2.7, 2.3, -2.3, 0.4999, -0.4999, 100.5, -100.5,
     7.5, -7.5, 8.5, -8.5, 1.0, -1.0, 0.0, 12.25,
     -12.25, 3.49999, -3.49999, 255.5, -255.5, 1023.5, 33.5, -33.5],
    np.float32)


def make_consts(sqrt_breaks, sqrt_slopes, sqrt_intercepts,
                recip_breaks, recip_slopes, recip_intercepts):
    c = np.zeros((1, CONST_W), np.float32)
    c[0, _SB:_SB + 33] = sqrt_breaks
    c[0, _SS:_SS + 32] = sqrt_slopes
    c[0, _SI:_SI + 32] = sqrt_intercepts
    c[0, _RB:_RB + 33] = recip_breaks
    c[0, _RS:_RS + 32] = recip_slopes
    c[0, _RI:_RI + 32] = recip_intercepts
    c[0, _IOTA:_IOTA + 32] = np.arange(32, dtype=np.float32)
    c[0, _PROBE:_PROBE + N_PROBE] = _PROBE_VALS
    return c


_NC_CACHE = {}


def _get_nc(rows_per_core, trivial):
    key = (rows_per_core, trivial)
    if key not in _NC_CACHE:
        _NC_CACHE[key] = build_nc(rows_per_core, trivial)
    return _NC_CACHE[key]


def run(x, weight, bias, consts, trace=False, **trace_kwargs):
    rows = x.shape[0] // N_CORES
    weight = np.asarray(weight, np.float32).reshape(1, D)
    bias = np.asarray(bias, np.float32).reshape(1, D)
    trivial = bool(np.all(weight == 1.0) and np.all(bias == 0.0))
    nc = _get_nc(rows, trivial)
    in_maps = []
    for i in range(N_CORES):
        m = {"x": np.ascontiguousarray(x[i * rows:(i + 1) * rows],
                                       dtype=np.float32),
             "consts": consts}
        if not trivial:
            m["weight"] = weight
            m["bias"] = bias
        in_maps.append(m)
    res = run_bass_kernel_spmd(nc, in_maps, core_ids=list(range(N_CORES)),
                               trace=trace, **trace_kwargs)
    out = np.concatenate([r["out"] for r in res.results], axis=0)
    return out, res


def kernel(x, weight, bias, sqrt_breaks, sqrt_slopes, sqrt_intercepts,
           recip_breaks, recip_slopes, recip_intercepts):
    x = np.asarray(x, dtype=np.float32)
    consts = make_consts(np.asarray(sqrt_breaks), np.asarray(sqrt_slopes),
                         np.asarray(sqrt_intercepts), np.asarray(recip_breaks),
                         np.asarray(recip_slopes), np.asarray(recip_intercepts))
    out, _ = run(x, np.asarray(weight), np.asarray(bias), consts, trace=False)
    return out
